# revision 10
# baseline (speedup 1.0000x reference)
"""Distributed multi-head attention kernel for 8 TRN2 NeuronCores.

Problem: x[4,2048,1024] -> qkv proj (w_qkv[3072,1024]) -> 16-head attention
         -> out proj (w_out[1024,1024], b_out) -> [4,2048,1024]

Sharding: core c handles batch b=c//2 and heads (c%2)*8 .. (c%2)*8+8
(data parallel over batch x tensor parallel over heads). Each core writes
its full [1024, 2048] output-projection partial (with half the bias) to
DRAM; the host sums the two partials of each batch during unshard, so no
on-device collective is needed and the last chunk has no reduce tail.

Per-core compute (bf16 matmuls, fp32 PSUM):
  Scores are computed transposed, S^T [j, i], two heads packed in PE
  row groups 0/64 (concurrent row-tiled pair). exp runs on ScalarE with
  the 1/sqrt(d) scale folded in. Softmax denominators come from
  ones-matmul column sums packed in PE col groups 0/64 so each head's
  denominator lands replicated under its own output partitions. O^T
  accumulates in PSUM (V stationary, col groups 0/64); VectorE applies
  a fast reciprocal and normalizes into mergedT (bf16), which feeds
  the output projection as the moving operand.

  The whole kernel is one software-pipelined stream: a 28-unit head
  start (K m0/m1, Q m0/m1 + Q(2,0)/Q(3,0), V all), then 4x64 attention
  units. Outstanding projection work (remaining K tiles, next chunk's Q,
  previous chunk's output projection) is injected at pr-segment
  boundaries, where the psA/psB PSUM rings have a free buffer, so the
  tensor engine never drains while ScalarE's exp stream stays hot.
  PSUM: 2x2 score banks + po + pd (bufs=2) = 8 banks.
"""

import numpy as np
import ml_dtypes

import concourse.bass as bass
import concourse.mybir as mybir
import concourse.tile as tile
from concourse import bacc
from concourse.bass_utils import run_bass_kernel_spmd

B, N, H = 4, 2048, 1024
NH, DH = 16, 64
NCORES = 8
HH = 512          # head dims per core (8 heads x 64)
KH = H // 128     # 8 hidden k-tiles
NJT = N // 128    # 16 token j-tiles
NCK = N // 512    # 4 token chunks
MT = HH // 128    # 4 head-dim partition tiles per core
SCALE = DH ** -0.5

BF16 = mybir.dt.bfloat16
F32 = mybir.dt.float32
Exp = mybir.ActivationFunctionType.Exp

_cache = {}


def _build():
    nc = bacc.Bacc(
        "TRN2", target_bir_lowering=False, debug=False, num_devices=NCORES
    )
    xT = nc.dram_tensor("xT", [H, N], BF16, kind="ExternalInput").ap()
    wqT = nc.dram_tensor("wqT", [H, HH], BF16, kind="ExternalInput").ap()
    wkT = nc.dram_tensor("wkT", [H, HH], BF16, kind="ExternalInput").ap()
    wvT = nc.dram_tensor("wvT", [H, HH], BF16, kind="ExternalInput").ap()
    woT = nc.dram_tensor("woT", [HH, H], BF16, kind="ExternalInput").ap()
    hbT = nc.dram_tensor("hbT", [128, KH], F32, kind="ExternalInput").ap()
    out_e = nc.dram_tensor("out", [H, N], F32, kind="ExternalOutput").ap()

    with tile.TileContext(nc) as tc:
        with (
            tc.tile_pool(name="singles", bufs=1) as singles,
            tc.tile_pool(name="psA", bufs=2, space="PSUM") as psA,
            tc.tile_pool(name="psB", bufs=2, space="PSUM") as psB,
            tc.tile_pool(name="stps", bufs=2, space="PSUM") as stps,
            tc.tile_pool(name="pe", bufs=12) as pe_pool,
            tc.tile_pool(name="rsb", bufs=4) as r_pool,
            tc.tile_pool(name="osb", bufs=6) as osb_pool,
        ):
            x_sb = singles.tile([128, KH, N], BF16)
            wq_sb = singles.tile([128, KH, HH], BF16)
            wk_sb = singles.tile([128, KH, HH], BF16)
            wv_sb = singles.tile([128, KH, HH], BF16)
            wo_sb = singles.tile([128, MT, H], BF16)
            hb_sb = singles.tile([128, KH], F32)
            qT_sb = singles.tile([128, MT, N], BF16)
            kT_sb = singles.tile([128, MT, N], BF16)
            v_sb = singles.tile([128, NJT, 8, DH], BF16)
            mT_sb = singles.tile([128, MT, N], BF16)
            ones_sb = singles.tile([128, DH], BF16)

            nc.vector.memset(ones_sb, 1.0)
            # split DMA issue across the Sync, Scalar and GpSimd queues
            # (~0.6us per issue): wk+x first so the opening K projection
            # starts as early as possible
            for k in range(KH):
                q = nc.sync if k % 2 == 0 else nc.scalar
                q.dma_start(out=wk_sb[:, k, :], in_=wkT[k * 128:(k + 1) * 128, :])
                q2 = nc.gpsimd if k % 2 == 0 else nc.sync
                q2.dma_start(out=x_sb[:, k, :], in_=xT[k * 128:(k + 1) * 128, :])
            for k in range(KH):
                q = nc.scalar if k % 2 == 0 else nc.sync
                q.dma_start(out=wq_sb[:, k, :], in_=wqT[k * 128:(k + 1) * 128, :])
                nc.gpsimd.dma_start(out=wv_sb[:, k, :], in_=wvT[k * 128:(k + 1) * 128, :])
            for m in range(MT):
                nc.gpsimd.dma_start(out=wo_sb[:, m, :], in_=woT[m * 128:(m + 1) * 128, :])
            nc.gpsimd.dma_start(out=hb_sb, in_=hbT)

            def kq_proj(w_sb, dst, m, ci, tg):
                pool = psA if tg == 0 else psB
                ps = pool.tile([128, 512], F32, tag="a" if tg == 0 else "b",
                               name=f"kq{m}_{ci}")
                for k in range(KH):
                    nc.tensor.matmul(
                        ps,
                        lhsT=w_sb[:, k, m * 128:(m + 1) * 128],
                        rhs=x_sb[:, k, ci * 512:(ci + 1) * 512],
                        start=(k == 0), stop=(k == KH - 1),
                    )
                nc.vector.tensor_copy(out=dst[:, m, ci * 512:(ci + 1) * 512], in_=ps)

            def kq_chunks(w_sb, dst, m, ci, tg):
                """kq_proj split into two 4-matmul chunks sharing one
                PSUM tile, so it can be drip-fed between attention
                units without starving ScalarE."""
                cell = {}

                def chunk(k0, k1):
                    def run():
                        if k0 == 0:
                            pool = psA if tg == 0 else psB
                            cell["ps"] = pool.tile(
                                [128, 512], F32,
                                tag="a" if tg == 0 else "b",
                                name=f"kqc{m}_{ci}",
                            )
                        ps = cell["ps"]
                        for k in range(k0, k1):
                            nc.tensor.matmul(
                                ps,
                                lhsT=w_sb[:, k, m * 128:(m + 1) * 128],
                                rhs=x_sb[:, k, ci * 512:(ci + 1) * 512],
                                start=(k == 0), stop=(k == KH - 1),
                                skip_group_check=True,
                            )
                        if k1 == KH:
                            nc.vector.tensor_copy(
                                out=dst[:, m, ci * 512:(ci + 1) * 512],
                                in_=ps,
                            )
                    return run

                return chunk(0, KH // 2), chunk(KH // 2, KH)

            def v_proj(jt, tg):
                pool = psA if tg == 0 else psB
                ps = pool.tile([128, 512], F32, tag="a" if tg == 0 else "b",
                               name=f"vp{jt}")
                for k in range(KH):
                    nc.tensor.matmul(
                        ps,
                        lhsT=x_sb[:, k, jt * 128:(jt + 1) * 128],
                        rhs=wv_sb[:, k, :],
                        start=(k == 0), stop=(k == KH - 1),
                    )
                nc.vector.tensor_copy(
                    out=v_sb[:, jt, :, :],
                    in_=ps.rearrange("p (h d) -> p h d", h=8),
                )

            def outproj_unit(ci, m, tg):
                pool = psA if tg == 0 else psB
                pp = pool.tile([128, 512], F32,
                               tag="a" if tg == 0 else "b",
                               name=f"pp{ci}_{m}")
                for kd in range(MT):
                    nc.tensor.matmul(
                        pp,
                        lhsT=wo_sb[:, kd, m * 128:(m + 1) * 128],
                        rhs=mT_sb[:, kd, ci * 512:(ci + 1) * 512],
                        start=(kd == 0), stop=(kd == MT - 1),
                    )
                ob = osb_pool.tile([128, 512], F32, tag="ob")
                nc.vector.tensor_scalar_add(
                    out=ob, in0=pp, scalar1=hb_sb[:, m:m + 1]
                )
                q = nc.sync if m % 2 == 0 else nc.gpsimd
                q.dma_start(
                    out=out_e[m * 128:(m + 1) * 128, ci * 512:(ci + 1) * 512],
                    in_=ob,
                )

            # ---- prologue head start: K m0/m1 (all token chunks),
            # Q(0..3, ci0), V all.  28 units, psA/psB ping-pong.
            for m in (0, 1):
                for ci in range(NCK):
                    kq_proj(wk_sb, kT_sb, m, ci, ci % 2)
            kq_proj(wq_sb, qT_sb, 0, 0, 0)
            kq_proj(wq_sb, qT_sb, 1, 0, 1)
            for jt in range(NJT):
                v_proj(jt, jt % 2)
            kq_proj(wq_sb, qT_sb, 2, 0, 0)
            kq_proj(wq_sb, qT_sb, 3, 0, 1)

            def attention_ci(ci, unit_sched):
                """64 attention units, processed in batches of two so the
                PE alternates row-mode (scores) and col-mode (pv+den)
                once per pair instead of once per unit.  unit_sched maps
                a unit index to small injected PE tasks (<=8 matmuls)
                that run right after that unit, where the psA/psB rings
                have a free buffer and ScalarE has 2 buffered exps."""
                units = [(pr, jt) for pr in range(4) for jt in range(NJT)]
                st_tiles = {}

                def emit_scores(u):
                    pr, jt = units[u]
                    st = stps.tile([128, 2, 512], F32, tag="st",
                                   name=f"st{ci}_{u}")
                    nc.tensor.matmul(
                        st[:, 0, :],
                        lhsT=kT_sb[0:64, pr, jt * 128:(jt + 1) * 128],
                        rhs=qT_sb[0:64, pr, ci * 512:(ci + 1) * 512],
                        start=True, stop=True,
                    )
                    nc.tensor.matmul(
                        st[:, 1, :],
                        lhsT=kT_sb[64:128, pr, jt * 128:(jt + 1) * 128],
                        rhs=qT_sb[64:128, pr, ci * 512:(ci + 1) * 512],
                        start=True, stop=True,
                    )
                    st_tiles[u] = st

                def emit_pv(po, pd, pe, pr, jt, first, last):
                    h0, h1 = 2 * pr, 2 * pr + 1
                    nc.tensor.matmul(
                        po[0:64, :], lhsT=v_sb[:, jt, h0, :], rhs=pe[:, 0, :],
                        start=first, stop=last, skip_group_check=True,
                    )
                    nc.tensor.matmul(
                        po[64:128, :], lhsT=v_sb[:, jt, h1, :], rhs=pe[:, 1, :],
                        start=first, stop=last, skip_group_check=True,
                    )
                    nc.tensor.matmul(
                        pd[0:64, :], lhsT=ones_sb, rhs=pe[:, 0, :],
                        start=first, stop=last, skip_group_check=True,
                    )
                    nc.tensor.matmul(
                        pd[64:128, :], lhsT=ones_sb, rhs=pe[:, 1, :],
                        start=first, stop=last, skip_group_check=True,
                    )

                emit_scores(0)
                emit_scores(1)
                po = pd = None
                for ub in range(0, len(units), 2):
                    # both exps first (frees both st buffers), then both
                    # score pairs in one row-mode visit, then the four
                    # col-mode pv/den pairs
                    pes = []
                    for u in (ub, ub + 1):
                        pr, jt = units[u]
                        if jt == 0:
                            po = psA.tile([128, 512], F32, tag="a",
                                          name=f"po{pr}")
                            pd = psB.tile([128, 512], F32, tag="b",
                                          name=f"pd{pr}")
                        pe = pe_pool.tile([128, 2, 512], BF16, tag="pe")
                        nc.scalar.activation(out=pe, in_=st_tiles.pop(u),
                                             func=Exp, scale=SCALE)
                        pes.append(pe)
                    if ub + 2 < len(units):
                        emit_scores(ub + 2)
                    if ub + 3 < len(units):
                        emit_scores(ub + 3)
                    for u, pe in zip((ub, ub + 1), pes):
                        pr, jt = units[u]
                        emit_pv(po, pd, pe, pr, jt,
                                jt == 0, jt == NJT - 1)
                        if jt == NJT - 1:
                            r = r_pool.tile([128, 512], F32, tag="r")
                            nc.vector.reciprocal_approx_fast(out=r, in_=pd)
                            nc.vector.tensor_mul(
                                out=mT_sb[:, pr, ci * 512:(ci + 1) * 512],
                                in0=po, in1=r,
                            )
                    for u in (ub, ub + 1):
                        for task in unit_sched.get(u, ()):
                            task()

            # ---- boundary task schedules -------------------------------
            # ci0: remaining K tiles (m2/m3).  ci>=1: next-chunk Q and the
            # previous chunk's output projection (2 m-units per pool ring
            # per boundary at most).
            def mk(f, *a):
                return lambda: f(*a)

            # Per-unit injection slots.  PSUM ring discipline (bufs=2,
            # strict round-robin): each pool supports ONE mid-segment
            # injected tile per pr segment, plus a chained tile right at
            # the boundary (where the long-lived po/pd has just freed).
            # Mid tiles are 4-matmul chunks so ScalarE's two buffered
            # exps cover the insertion; boundary chains are whole 8-MM
            # kq units (psB, freed by the reciprocal a bit earlier).
            schedules = {ci: {} for ci in range(NCK)}

            def put(ci, u, task):
                schedules[ci].setdefault(u, []).append(task)

            # ci0: K m2/m3 as mid-segment chunk pairs, Q(*,1) at
            # boundaries.  psA mids at 16k+1/16k+5, psB at 16k+3/16k+7.
            k_specs = [(2, 0), (2, 1), (2, 2), (2, 3),
                       (3, 0), (3, 1), (3, 2), (3, 3)]
            for seg in range(4):
                ma, mc = k_specs[2 * seg], k_specs[2 * seg + 1]
                ca, cb = kq_chunks(wk_sb, kT_sb, ma[0], ma[1], 0)
                put(0, 16 * seg + 1, ca)
                put(0, 16 * seg + 5, cb)
                ca, cb = kq_chunks(wk_sb, kT_sb, mc[0], mc[1], 1)
                put(0, 16 * seg + 3, ca)
                put(0, 16 * seg + 7, cb)
                put(0, 16 * seg + 15,
                    mk(kq_proj, wq_sb, qT_sb, seg, 1, 1))
            # steady chunks: outproj mids, next-chunk Q at boundaries
            for ci in range(1, NCK):
                for seg in range(4):
                    put(ci, 16 * seg + 5,
                        mk(outproj_unit, ci - 1, 2 * seg, 0))
                    put(ci, 16 * seg + 9,
                        mk(outproj_unit, ci - 1, 2 * seg + 1, 1))
                    if ci + 1 < NCK:
                        put(ci, 16 * seg + 15,
                            mk(kq_proj, wq_sb, qT_sb, seg, ci + 1, 1))

            for ci in range(NCK):
                attention_ci(ci, schedules[ci])
            # tail: last chunk's output projection
            for m in range(8):
                outproj_unit(NCK - 1, m, m % 2)

    nc.compile()
    return nc


def _get_nc():
    if "nc" not in _cache:
        _cache["nc"] = _build()
    return _cache["nc"]


def _shard_inputs(x, w_qkv, w_out, b_out):
    bf16 = ml_dtypes.bfloat16
    in_maps = []
    for c in range(NCORES):
        b, hh = c // 2, c % 2
        r0 = hh * HH
        hbT = (0.5 * b_out).astype(np.float32).reshape(KH, 128).T
        in_maps.append({
            "xT": np.ascontiguousarray(x[b].T).astype(bf16),
            "wqT": np.ascontiguousarray(w_qkv[r0:r0 + HH, :].T).astype(bf16),
            "wkT": np.ascontiguousarray(w_qkv[H + r0:H + r0 + HH, :].T).astype(bf16),
            "wvT": np.ascontiguousarray(w_qkv[2 * H + r0:2 * H + r0 + HH, :].T).astype(bf16),
            "woT": np.ascontiguousarray(w_out[:, r0:r0 + HH].T).astype(bf16),
            "hbT": np.ascontiguousarray(hbT),
        })
    return in_maps


def _assemble(results):
    out = np.empty((B, N, H), dtype=np.float32)
    for b in range(B):
        lo = np.asarray(results[2 * b]["out"])
        hi = np.asarray(results[2 * b + 1]["out"])
        out[b] = (lo + hi).T
    return out


def run_sharded(x, w_qkv, w_out, b_out, trace=False):
    nc = _get_nc()
    in_maps = _shard_inputs(x, w_qkv, w_out, b_out)
    res = run_bass_kernel_spmd(nc, in_maps, core_ids=list(range(NCORES)),
                               trace=trace)
    return _assemble(res.results), res


def kernel(x, w_qkv, w_out, b_out):
    x = np.asarray(x, dtype=np.float32)
    w_qkv = np.asarray(w_qkv, dtype=np.float32)
    w_out = np.asarray(w_out, dtype=np.float32)
    b_out = np.asarray(b_out, dtype=np.float32)
    out, _ = run_sharded(x, w_qkv, w_out, b_out, trace=False)
    return out


# revision 13
# speedup vs baseline: 1.0251x; 1.0251x over previous
"""Distributed multi-head attention kernel for 8 TRN2 NeuronCores.

Problem: x[4,2048,1024] -> qkv proj (w_qkv[3072,1024]) -> 16-head attention
         -> out proj (w_out[1024,1024], b_out) -> [4,2048,1024]

Sharding: core c handles batch b=c//2 and heads (c%2)*8 .. (c%2)*8+8
(data parallel over batch x tensor parallel over heads). Each core writes
its full [1024, 2048] output-projection partial (with half the bias) to
DRAM; the host sums the two partials of each batch during unshard, so no
on-device collective is needed and the last chunk has no reduce tail.

Per-core compute (bf16 matmuls, fp32 PSUM):
  Scores are computed transposed, S^T [j, i], two heads packed in PE
  row groups 0/64 (concurrent row-tiled pair). exp runs on ScalarE with
  the 1/sqrt(d) scale folded in. Softmax denominators come from
  ones-matmul column sums packed in PE col groups 0/64 so each head's
  denominator lands replicated under its own output partitions. O^T
  accumulates in PSUM (V stationary, col groups 0/64); VectorE applies
  a fast reciprocal and normalizes into mergedT (bf16), which feeds
  the output projection as the moving operand.

  The whole kernel is one software-pipelined stream: a 28-unit head
  start (K m0/m1, Q m0/m1 + Q(2,0)/Q(3,0), V all), then 4x64 attention
  units. Outstanding projection work (remaining K tiles, next chunk's Q,
  previous chunk's output projection) is injected at pr-segment
  boundaries, where the psA/psB PSUM rings have a free buffer, so the
  tensor engine never drains while ScalarE's exp stream stays hot.
  PSUM: 2x2 score banks + po + pd (bufs=2) = 8 banks.
"""

import numpy as np
import ml_dtypes

import concourse.bass as bass
import concourse.mybir as mybir
import concourse.tile as tile
from concourse import bacc
from concourse.bass_utils import run_bass_kernel_spmd

B, N, H = 4, 2048, 1024
NH, DH = 16, 64
NCORES = 8
HH = 512          # head dims per core (8 heads x 64)
KH = H // 128     # 8 hidden k-tiles
NJT = N // 128    # 16 token j-tiles
NCK = N // 512    # 4 token chunks
MT = HH // 128    # 4 head-dim partition tiles per core
SCALE = DH ** -0.5

BF16 = mybir.dt.bfloat16
F32 = mybir.dt.float32
Exp = mybir.ActivationFunctionType.Exp

_cache = {}


def _build():
    nc = bacc.Bacc(
        "TRN2", target_bir_lowering=False, debug=False, num_devices=NCORES
    )
    xT = nc.dram_tensor("xT", [H, N], BF16, kind="ExternalInput").ap()
    wqT = nc.dram_tensor("wqT", [H, HH], BF16, kind="ExternalInput").ap()
    wkT = nc.dram_tensor("wkT", [H, HH], BF16, kind="ExternalInput").ap()
    wvT = nc.dram_tensor("wvT", [H, HH], BF16, kind="ExternalInput").ap()
    woT = nc.dram_tensor("woT", [HH, H], BF16, kind="ExternalInput").ap()
    hbT = nc.dram_tensor("hbT", [128, KH], F32, kind="ExternalInput").ap()
    out_e = nc.dram_tensor("out", [H, N], F32, kind="ExternalOutput").ap()

    with tile.TileContext(nc) as tc:
        with (
            tc.tile_pool(name="singles", bufs=1) as singles,
            tc.tile_pool(name="psA", bufs=2, space="PSUM") as psA,
            tc.tile_pool(name="psB", bufs=2, space="PSUM") as psB,
            tc.tile_pool(name="stps", bufs=2, space="PSUM") as stps,
            tc.tile_pool(name="pe", bufs=12) as pe_pool,
            tc.tile_pool(name="rsb", bufs=4) as r_pool,
            tc.tile_pool(name="osb", bufs=6) as osb_pool,
        ):
            x_sb = singles.tile([128, KH, N], BF16)
            wq_sb = singles.tile([128, KH, HH], BF16)
            wk_sb = singles.tile([128, KH, HH], BF16)
            wv_sb = singles.tile([128, KH, HH], BF16)
            wo_sb = singles.tile([128, MT, H], BF16)
            hb_sb = singles.tile([128, KH], F32)
            qT_sb = singles.tile([128, MT, N], BF16)
            kT_sb = singles.tile([128, MT, N], BF16)
            v_sb = singles.tile([128, NJT, 8, DH], BF16)
            mT_sb = singles.tile([128, MT, N], BF16)
            ones_sb = singles.tile([128, DH], BF16)

            nc.vector.memset(ones_sb, 1.0)
            # split DMA issue across the Sync, Scalar and GpSimd queues
            # (~0.6us per issue): wk+x first so the opening K projection
            # starts as early as possible
            for k in range(KH):
                nc.sync.dma_start(out=wk_sb[:, k, :], in_=wkT[k * 128:(k + 1) * 128, :])
                nc.gpsimd.dma_start(out=x_sb[:, k, :], in_=xT[k * 128:(k + 1) * 128, :])
            for k in range(KH):
                nc.scalar.dma_start(out=wq_sb[:, k, :], in_=wqT[k * 128:(k + 1) * 128, :])
                nc.gpsimd.dma_start(out=wv_sb[:, k, :], in_=wvT[k * 128:(k + 1) * 128, :])
            for m in range(MT):
                nc.gpsimd.dma_start(out=wo_sb[:, m, :], in_=woT[m * 128:(m + 1) * 128, :])
            nc.gpsimd.dma_start(out=hb_sb, in_=hbT)

            def kq_proj(w_sb, dst, m, ci, tg):
                pool = psA if tg == 0 else psB
                ps = pool.tile([128, 512], F32, tag="a" if tg == 0 else "b",
                               name=f"kq{m}_{ci}")
                for k in range(KH):
                    nc.tensor.matmul(
                        ps,
                        lhsT=w_sb[:, k, m * 128:(m + 1) * 128],
                        rhs=x_sb[:, k, ci * 512:(ci + 1) * 512],
                        start=(k == 0), stop=(k == KH - 1),
                    )
                nc.vector.tensor_copy(out=dst[:, m, ci * 512:(ci + 1) * 512], in_=ps)

            def kq_chunks(w_sb, dst, m, ci, tg):
                """kq_proj split into two 4-matmul chunks sharing one
                PSUM tile, so it can be drip-fed between attention
                units without starving ScalarE."""
                cell = {}

                def chunk(k0, k1):
                    def run():
                        if k0 == 0:
                            pool = psA if tg == 0 else psB
                            cell["ps"] = pool.tile(
                                [128, 512], F32,
                                tag="a" if tg == 0 else "b",
                                name=f"kqc{m}_{ci}",
                            )
                        ps = cell["ps"]
                        for k in range(k0, k1):
                            nc.tensor.matmul(
                                ps,
                                lhsT=w_sb[:, k, m * 128:(m + 1) * 128],
                                rhs=x_sb[:, k, ci * 512:(ci + 1) * 512],
                                start=(k == 0), stop=(k == KH - 1),
                                skip_group_check=True,
                            )
                        if k1 == KH:
                            nc.vector.tensor_copy(
                                out=dst[:, m, ci * 512:(ci + 1) * 512],
                                in_=ps,
                            )
                    return run

                return chunk(0, KH // 2), chunk(KH // 2, KH)

            def v_proj(jt, tg):
                pool = psA if tg == 0 else psB
                ps = pool.tile([128, 512], F32, tag="a" if tg == 0 else "b",
                               name=f"vp{jt}")
                for k in range(KH):
                    nc.tensor.matmul(
                        ps,
                        lhsT=x_sb[:, k, jt * 128:(jt + 1) * 128],
                        rhs=wv_sb[:, k, :],
                        start=(k == 0), stop=(k == KH - 1),
                    )
                nc.vector.tensor_copy(
                    out=v_sb[:, jt, :, :],
                    in_=ps.rearrange("p (h d) -> p h d", h=8),
                )

            def outproj_unit(ci, m, tg):
                pool = psA if tg == 0 else psB
                pp = pool.tile([128, 512], F32,
                               tag="a" if tg == 0 else "b",
                               name=f"pp{ci}_{m}")
                for kd in range(MT):
                    nc.tensor.matmul(
                        pp,
                        lhsT=wo_sb[:, kd, m * 128:(m + 1) * 128],
                        rhs=mT_sb[:, kd, ci * 512:(ci + 1) * 512],
                        start=(kd == 0), stop=(kd == MT - 1),
                    )
                ob = osb_pool.tile([128, 512], F32, tag="ob")
                nc.vector.tensor_scalar_add(
                    out=ob, in0=pp, scalar1=hb_sb[:, m:m + 1]
                )
                q = nc.sync if m % 2 == 0 else nc.gpsimd
                q.dma_start(
                    out=out_e[m * 128:(m + 1) * 128, ci * 512:(ci + 1) * 512],
                    in_=ob,
                )

            # ---- prologue head start: K m0/m1 (all token chunks),
            # Q(0..3, ci0), V all.  28 units, psA/psB ping-pong.
            for m in (0, 1):
                for ci in range(NCK):
                    kq_proj(wk_sb, kT_sb, m, ci, ci % 2)
            kq_proj(wq_sb, qT_sb, 0, 0, 0)
            kq_proj(wq_sb, qT_sb, 1, 0, 1)
            for jt in range(NJT):
                v_proj(jt, jt % 2)
            kq_proj(wq_sb, qT_sb, 2, 0, 0)
            kq_proj(wq_sb, qT_sb, 3, 0, 1)

            def attention_ci(ci, unit_sched):
                """64 attention units, processed in batches of two so the
                PE alternates row-mode (scores) and col-mode (pv+den)
                once per pair instead of once per unit.  unit_sched maps
                a unit index to small injected PE tasks (<=8 matmuls)
                that run right after that unit, where the psA/psB rings
                have a free buffer and ScalarE has 2 buffered exps."""
                units = [(pr, jt) for pr in range(4) for jt in range(NJT)]
                st_tiles = {}

                def emit_scores(u):
                    pr, jt = units[u]
                    st = stps.tile([128, 2, 512], F32, tag="st",
                                   name=f"st{ci}_{u}")
                    nc.tensor.matmul(
                        st[:, 0, :],
                        lhsT=kT_sb[0:64, pr, jt * 128:(jt + 1) * 128],
                        rhs=qT_sb[0:64, pr, ci * 512:(ci + 1) * 512],
                        start=True, stop=True,
                    )
                    nc.tensor.matmul(
                        st[:, 1, :],
                        lhsT=kT_sb[64:128, pr, jt * 128:(jt + 1) * 128],
                        rhs=qT_sb[64:128, pr, ci * 512:(ci + 1) * 512],
                        start=True, stop=True,
                    )
                    st_tiles[u] = st

                def emit_pv(po, pd, pe, pr, jt, first, last):
                    h0, h1 = 2 * pr, 2 * pr + 1
                    nc.tensor.matmul(
                        po[0:64, :], lhsT=v_sb[:, jt, h0, :], rhs=pe[:, 0, :],
                        start=first, stop=last, skip_group_check=True,
                    )
                    nc.tensor.matmul(
                        po[64:128, :], lhsT=v_sb[:, jt, h1, :], rhs=pe[:, 1, :],
                        start=first, stop=last, skip_group_check=True,
                    )
                    nc.tensor.matmul(
                        pd[0:64, :], lhsT=ones_sb, rhs=pe[:, 0, :],
                        start=first, stop=last, skip_group_check=True,
                    )
                    nc.tensor.matmul(
                        pd[64:128, :], lhsT=ones_sb, rhs=pe[:, 1, :],
                        start=first, stop=last, skip_group_check=True,
                    )

                emit_scores(0)
                emit_scores(1)
                po = pd = None
                for ub in range(0, len(units), 2):
                    # both exps first (frees both st buffers), then both
                    # score pairs in one row-mode visit, then the four
                    # col-mode pv/den pairs
                    pes = []
                    for u in (ub, ub + 1):
                        pr, jt = units[u]
                        if jt == 0:
                            po = psA.tile([128, 512], F32, tag="a",
                                          name=f"po{pr}")
                            pd = psB.tile([128, 512], F32, tag="b",
                                          name=f"pd{pr}")
                        pe = pe_pool.tile([128, 2, 512], BF16, tag="pe")
                        nc.scalar.activation(out=pe, in_=st_tiles.pop(u),
                                             func=Exp, scale=SCALE)
                        pes.append(pe)
                    if ub + 2 < len(units):
                        emit_scores(ub + 2)
                    if ub + 3 < len(units):
                        emit_scores(ub + 3)
                    for u, pe in zip((ub, ub + 1), pes):
                        pr, jt = units[u]
                        emit_pv(po, pd, pe, pr, jt,
                                jt == 0, jt == NJT - 1)
                        if jt == NJT - 1:
                            r = r_pool.tile([128, 512], F32, tag="r")
                            nc.vector.reciprocal_approx_fast(out=r, in_=pd)
                            nc.vector.tensor_mul(
                                out=mT_sb[:, pr, ci * 512:(ci + 1) * 512],
                                in0=po, in1=r,
                            )
                    for u in (ub, ub + 1):
                        for task in unit_sched.get(u, ()):
                            task()

            # ---- boundary task schedules -------------------------------
            # ci0: remaining K tiles (m2/m3).  ci>=1: next-chunk Q and the
            # previous chunk's output projection (2 m-units per pool ring
            # per boundary at most).
            def mk(f, *a):
                return lambda: f(*a)

            # Per-unit injection slots.  PSUM ring discipline (bufs=2,
            # strict round-robin): each pool supports ONE mid-segment
            # injected tile per pr segment, plus a chained tile right at
            # the boundary (where the long-lived po/pd has just freed).
            # Mid tiles are 4-matmul chunks so ScalarE's two buffered
            # exps cover the insertion; boundary chains are whole 8-MM
            # kq units (psB, freed by the reciprocal a bit earlier).
            schedules = {ci: {} for ci in range(NCK)}

            def put(ci, u, task):
                schedules[ci].setdefault(u, []).append(task)

            # ci0: K m2/m3 as mid-segment chunk pairs, Q(*,1) at
            # boundaries.  psA mids at 16k+1/16k+5, psB at 16k+3/16k+7.
            k_specs = [(2, 0), (2, 1), (2, 2), (2, 3),
                       (3, 0), (3, 1), (3, 2), (3, 3)]
            for seg in range(4):
                ma, mc = k_specs[2 * seg], k_specs[2 * seg + 1]
                ca, cb = kq_chunks(wk_sb, kT_sb, ma[0], ma[1], 0)
                put(0, 16 * seg + 1, ca)
                put(0, 16 * seg + 5, cb)
                ca, cb = kq_chunks(wk_sb, kT_sb, mc[0], mc[1], 1)
                put(0, 16 * seg + 3, ca)
                put(0, 16 * seg + 7, cb)
                qa, qb = kq_chunks(wq_sb, qT_sb, seg, 1, 1)
                if seg < 3:
                    put(0, 16 * seg + 15, qa)
                    put(0, 16 * seg + 17, qb)
                else:
                    put(0, 16 * seg + 15,
                        mk(kq_proj, wq_sb, qT_sb, seg, 1, 1))
            # steady chunks: outproj mids, next-chunk Q at boundaries
            for ci in range(1, NCK):
                for seg in range(4):
                    put(ci, 16 * seg + 5,
                        mk(outproj_unit, ci - 1, 2 * seg, 0))
                    put(ci, 16 * seg + 9,
                        mk(outproj_unit, ci - 1, 2 * seg + 1, 1))
                    if ci + 1 < NCK:
                        qa, qb = kq_chunks(wq_sb, qT_sb, seg, ci + 1, 1)
                        if seg < 3:
                            put(ci, 16 * seg + 15, qa)
                            put(ci, 16 * seg + 17, qb)
                        else:
                            put(ci, 16 * seg + 15,
                                mk(kq_proj, wq_sb, qT_sb, seg, ci + 1, 1))

            for ci in range(NCK):
                attention_ci(ci, schedules[ci])
            # tail: last chunk's output projection
            for m in range(8):
                outproj_unit(NCK - 1, m, m % 2)

    nc.compile()
    return nc


def _get_nc():
    if "nc" not in _cache:
        _cache["nc"] = _build()
    return _cache["nc"]


def _shard_inputs(x, w_qkv, w_out, b_out):
    bf16 = ml_dtypes.bfloat16
    in_maps = []
    for c in range(NCORES):
        b, hh = c // 2, c % 2
        r0 = hh * HH
        hbT = (0.5 * b_out).astype(np.float32).reshape(KH, 128).T
        in_maps.append({
            "xT": np.ascontiguousarray(x[b].T).astype(bf16),
            "wqT": np.ascontiguousarray(w_qkv[r0:r0 + HH, :].T).astype(bf16),
            "wkT": np.ascontiguousarray(w_qkv[H + r0:H + r0 + HH, :].T).astype(bf16),
            "wvT": np.ascontiguousarray(w_qkv[2 * H + r0:2 * H + r0 + HH, :].T).astype(bf16),
            "woT": np.ascontiguousarray(w_out[:, r0:r0 + HH].T).astype(bf16),
            "hbT": np.ascontiguousarray(hbT),
        })
    return in_maps


def _assemble(results):
    out = np.empty((B, N, H), dtype=np.float32)
    for b in range(B):
        lo = np.asarray(results[2 * b]["out"])
        hi = np.asarray(results[2 * b + 1]["out"])
        out[b] = (lo + hi).T
    return out


def run_sharded(x, w_qkv, w_out, b_out, trace=False):
    nc = _get_nc()
    in_maps = _shard_inputs(x, w_qkv, w_out, b_out)
    res = run_bass_kernel_spmd(nc, in_maps, core_ids=list(range(NCORES)),
                               trace=trace)
    return _assemble(res.results), res


def kernel(x, w_qkv, w_out, b_out):
    x = np.asarray(x, dtype=np.float32)
    w_qkv = np.asarray(w_qkv, dtype=np.float32)
    w_out = np.asarray(w_out, dtype=np.float32)
    b_out = np.asarray(b_out, dtype=np.float32)
    out, _ = run_sharded(x, w_qkv, w_out, b_out, trace=False)
    return out
